# revision 58
# baseline (speedup 1.0000x reference)
"""Distributed multi-head attention for TRN2 (8 NeuronCores).

Reference computation (per batch b):
    qkv = x @ w_qkv.T                         # (N, 3C)
    q, k, v = split/reshape to (H, N, D)
    attn = softmax(q @ k.T * D**-0.5)         # per head
    out = (attn @ v) reassembled to (N, C)
    out = out @ w_proj.T + b_proj

Sharding: 8 cores = 4 batches x 2 HEAD-halves (tensor parallel). Each
core computes q/k/v for its own 6 heads over all 2048 tokens (no
duplicated projection work), full attention for those heads, and a
PARTIAL output projection over its 384 channels (+ bias/2). The host
sums the two partial projections of each batch - zero cross-core comm.

Layout strategy (no on-chip transposes):
  - host passes x^T and w_qkv^T slices so projections contract over
    partitions; q,k are produced d-major; scores are computed
    transposed ([keys, queries]) which is the layout attn@v consumes
  - softmax needs no max-subtraction (scores ~ N(0,1), fp32 exp range)
  - attn@v is COLUMN-TILED: the two heads of a pair run concurrently
    in the 128x128 PE array (head A -> output partitions 0:64, head B
    -> 64:128), doubling attn@v throughput vs a 65-wide ones-column
    form
  - softmax denominators come from 4-way column-tiled ones-matmuls
    (M=1 at col strips 0/32/64/96: head x kb-parity) accumulating in
    one PSUM bank across the pass
  - normalization: DVE sums the two strip partials per head, takes the
    reciprocal, a K=1 ones-matmul broadcasts 1/den across partitions,
    and one [128,512] DVE multiply normalizes both heads at once

Schedule: ACT (softmax exp, ~1.12us per 128x1024 tile) is the
steady-state bottleneck; the whole PE schedule is built to never let
it starve. 12 passes = 3 head pairs x 4 query quarters (pair-major),
SOFTWARE-PIPELINED ACROSS PASSES: each pass emits its first scores
before draining the previous pass's last attn@v/den and epilogue, so
the ACT queue never empties at pass boundaries. Per 2-kb step: scores
alternate PE row groups; attn@v + den lag one step. PE fillers are
sized to fit the per-step exp budget (k/q units split into 3-matmul
halves): pass 0 produces v just-in-time, passes 1-8 drain k/q blocks,
passes 9-11 run the partial output projection for completed quarters.
PSUM: scores 2x2 banks + acc 1 + den 1 + filler 2 = 8.

Self-contained: hardcodes B=4, N=2048, C=768, H=12, D=64.
"""

import numpy as np
import ml_dtypes

import concourse.bass as bass
import concourse.mybir as mybir
from concourse import bacc
from concourse.tile import TileContext
from concourse.bass_utils import run_bass_kernel_spmd

F32 = mybir.dt.float32
F16 = mybir.dt.float16
BF16 = mybir.dt.bfloat16
EXP = mybir.ActivationFunctionType.Exp

B, N, C = 4, 2048, 768
H, D = 12, 64
SCALE = float(D) ** -0.5  # 0.125
HC = H // 2  # 6 heads per core
CO = HC * D  # 384 own channels
HB = HC // 2  # 3 head pairs per core
CB = C // 128  # 6 input c-chunks
COB = CO // 128  # 3 own-channel chunks
TB = N // 128  # 16 token blocks
VW = CO  # 384: v block width

N_CORES = 8

# w_qkv^T column groups (own-head slice), in consumption order:
# pair-0 k, pair-0 q, all v, then k/q for pairs 1..2. Each group holds
# its column range for all six 128-row input chunks, contiguously.
# (o0 is the offset in the 3*CO-wide own-qkv column space: q [0,CO),
# k [CO,2CO), v [2CO,3CO).)
_WQ_GROUPS = [(CO, 128), (0, 128), (2 * CO, CO)]
for _ob in range(1, COB):
    _WQ_GROUPS.append((CO + _ob * 128, 128))
    _WQ_GROUPS.append((_ob * 128, 128))
_WQ_BASE = {}
_cur = 0
for _o0, _w in _WQ_GROUPS:
    _WQ_BASE[_o0] = (_cur, _w)
    _cur += CB * _w


def _build():
    nc = bacc.Bacc(None, target_bir_lowering=False)

    xTp = nc.declare_dram_parameter("xTp", [128, CB * N], BF16, isOutput=False)
    wqp = nc.declare_dram_parameter("wqp", [128, CB * 3 * CO], BF16, isOutput=False)
    wprojp = nc.declare_dram_parameter("wprojp", [128, COB * C], BF16, isOutput=False)
    outT = nc.declare_dram_parameter("outT", [C, N], F16, isOutput=True)

    with TileContext(nc) as tc:
        with (
            tc.tile_pool(name="per", bufs=1) as per,
            tc.tile_pool(name="p23", bufs=1) as p23,
            tc.tile_pool(name="hp", bufs=8) as hp,
            tc.tile_pool(name="mi", bufs=3) as mi,
            tc.tile_pool(name="op", bufs=2) as op_pool,
            tc.tile_pool(name="ps", bufs=2, space="PSUM") as ps2,
            tc.tile_pool(name="psa", bufs=1, space="PSUM") as ps1,
        ):
            # ---- persistent tiles -------------------------------------
            qT_sb = per.tile([128, HB * N], BF16)  # q^T  [2 heads/blk, 2048]
            kT_sb = per.tile([128, HB * N], BF16)  # k^T  [2 heads/blk, 2048]
            v_sb = per.tile([128, TB * VW], BF16)  # v token-major
            ones_sb = per.tile([128, 1], BF16)  # den stationary
            onesK1 = per.tile([1, 64], BF16)  # epi broadcast stationary
            warm_sb = per.tile([128, 512], BF16)  # PE warm-up operand
            attnT_sb = p23.tile([128, HB * N], BF16)  # attn out^T
            wproj_sb = p23.tile([128, COB * C], BF16)

            nc.vector.memset(ones_sb[:, :], 1.0)
            nc.vector.memset(onesK1[:, :], 1.0)
            nc.vector.memset(warm_sb[:, :], 0.0)

            def warm_pe(psum_ap, n):
                """dummy full-array matmuls that keep the PE's activity
                monitor from re-throttling the clock across known idle
                stretches (small-K matmuls don't register as busy)"""
                for _ in range(n):
                    nc.tensor.matmul(
                        psum_ap,
                        warm_sb[:, 0:128],
                        warm_sb[:, :],
                        start=True,
                        stop=True,
                    )

            wqxt = (tc.tile_pool(name="wq", bufs=1), tc.tile_pool(name="xt", bufs=4))
            wq_pool = wqxt[0].__enter__()
            xt_pool = wqxt[1].__enter__()

            wqkv_sb = wq_pool.tile([128, CB * 3 * CO], BF16)
            xts = [
                xt_pool.tile([128, CB * 512], BF16, tag="xt", name=f"xt{t}")
                for t in range(4)
            ]

            def _dma_xt(tch):
                nc.sync.dma_start(
                    out=xts[tch][:, :],
                    in_=xTp[:, tch * CB * 512 : (tch + 1) * CB * 512],
                )

            def _dma_wq(gi):
                o0, w = _WQ_GROUPS[gi]
                base, _ = _WQ_BASE[o0]
                nc.sync.dma_start(
                    out=wqkv_sb[:, base : base + CB * w],
                    in_=wqp[:, base : base + CB * w],
                )

            def _dma_xt0_chunk(c0, c1):
                nc.sync.dma_start(
                    out=xts[0][:, c0 * 512 : c1 * 512],
                    in_=xTp[:, c0 * 512 : c1 * 512],
                )

            # consumption order: pair-0 k/q cols, then chunk 0 in pieces
            # (the pre-phase k unit consumes ci in order, so compute can
            # start while the rest of chunk 0 is still in flight), v cols
            # (JIT v units in pass 0), remaining chunks, later k/q cols
            _dma_wq(0)
            _dma_wq(1)
            _dma_xt0_chunk(0, 2)
            _dma_xt0_chunk(2, 4)
            _dma_xt0_chunk(4, 6)
            _dma_wq(2)
            for t in range(1, 4):
                _dma_xt(t)
            for gi in range(3, len(_WQ_GROUPS)):
                _dma_wq(gi)

            def wq(ci, o0, width):
                if o0 >= 2 * CO:
                    base, gw = _WQ_BASE[2 * CO]
                    off = o0 - 2 * CO
                else:
                    base, gw = _WQ_BASE[o0]
                    off = 0
                return wqkv_sb[:, base + ci * gw + off : base + ci * gw + off + width]

            nc.sync.dma_start(out=wproj_sb[:, :], in_=wprojp[:, :])

            # ---- projection work units (PE filler) --------------------
            open_kq = {}

            def kq_half(hb, tch, is_q, half):
                """half of a k^T/q^T pair-block unit (3 of 6 matmuls)"""
                key = (hb, tch, is_q)
                if half == 0:
                    open_kq[key] = ps2.tile(
                        [128, 512], F32, tag="psV", bufs=2,
                        name=f"{'q' if is_q else 'k'}{hb}_{tch}",
                    )
                psv = open_kq[key]
                for ci in range(3 * half, 3 * half + 3):
                    nc.tensor.matmul(
                        psv[:, :],
                        wq(ci, (0 if is_q else CO) + hb * 128, 128),
                        xts[tch][:, ci * 512 : (ci + 1) * 512],
                        start=(ci == 0),
                        stop=(ci == CB - 1),
                    )
                if half == 1:
                    t0 = tch * 512
                    dst = qT_sb if is_q else kT_sb
                    nc.vector.tensor_copy(
                        dst[:, hb * N + t0 : hb * N + t0 + 512], psv[:, :]
                    )
                    del open_kq[key]

            def kq_tok(hb, tch, is_q, t0, w):
                """k^T/q^T unit over a token sub-range (shorter chain so
                the first scores aren't gated on a full 512-token unit)"""
                kind = "q" if is_q else "k"
                psv = ps2.tile(
                    [128, 512], F32, tag="psV", bufs=2,
                    name=f"{kind}t{hb}_{tch}_{t0}",
                )
                for ci in range(CB):
                    nc.tensor.matmul(
                        psv[:, :w],
                        wq(ci, (0 if is_q else CO) + hb * 128, 128),
                        xts[tch][:, ci * 512 + t0 : ci * 512 + t0 + w],
                        start=(ci == 0),
                        stop=(ci == CB - 1),
                    )
                dst = qT_sb if is_q else kT_sb
                nc.vector.tensor_copy(
                    dst[:, hb * N + tch * 512 + t0 : hb * N + tch * 512 + t0 + w],
                    psv[:, :w],
                )

            def v_unit(t128, part):
                """one v block: 128 tokens x one pair's 128 v-dims.
                Pair 0 is computed just-in-time in pass 0; pair 1 in
                passes 1-3 (needed by pass 4); pair 2 in passes 4-7
                (needed by pass 8)"""
                off, w = 128 * part, 128
                tch, tb = divmod(t128, 4)
                psv = ps2.tile(
                    [128, 512], F32, tag="psV", bufs=2, name=f"v{t128}_{part}"
                )
                for ci in range(CB):
                    nc.tensor.matmul(
                        psv[:, :w],
                        xts[tch][:, ci * 512 + tb * 128 : ci * 512 + (tb + 1) * 128],
                        wq(ci, 2 * CO + off, w),
                        start=(ci == 0),
                        stop=(ci == CB - 1),
                    )
                nc.vector.tensor_copy(
                    v_sb[:, t128 * VW + off : t128 * VW + off + w], psv[:, :w]
                )

            def proj_mms(ob, qc, psv, cbs):
                for cb in cbs:
                    nc.tensor.matmul(
                        psv[0:128, 0:512],
                        wproj_sb[:, cb * C + ob * 128 : cb * C + (ob + 1) * 128],
                        attnT_sb[:, cb * N + qc * 512 : cb * N + (qc + 1) * 512],
                        start=(cb == 0),
                        stop=(cb == COB - 1),
                    )

            def proj_drain(ob, qc, psv, on_act=False):
                # fp16 partial projection out; the host adds the bias
                # while summing the two head-half partials. In the tail
                # the scalar engine is idle (no more exps), so drains go
                # there instead of the DVE, which is busy with the final
                # epilogue chain.
                ot = op_pool.tile([128, 512], F16, tag="out")
                if on_act:
                    nc.scalar.copy(ot[:, :], psv[0:128, 0:512])
                else:
                    nc.vector.tensor_copy(ot[:, :], psv[0:128, 0:512])
                nc.sync.dma_start(
                    out=outT[ob * 128 : (ob + 1) * 128, qc * 512 : (qc + 1) * 512],
                    in_=ot[:, :],
                )

            def proj_unit(ob, qc, on_act=False):
                psv = ps2.tile([128, 512], F32, tag="psV", bufs=2, name=f"pr{ob}_{qc}")
                proj_mms(ob, qc, psv, range(COB))
                proj_drain(ob, qc, psv, on_act)

            # ---- attention machinery ----------------------------------
            def av_mms(acc, hb, pkb, ppb):
                for hh in range(2):
                    vs = pkb * VW + (2 * hb + hh) * D
                    nc.tensor.matmul(
                        acc[64 * hh : 64 * hh + 64, :],
                        v_sb[:, vs : vs + D],
                        ppb[:, hh * 512 : (hh + 1) * 512],
                        start=(pkb == 0),
                        stop=(pkb == TB - 1),
                        tile_position=(0, 64 * hh),
                    )

            def den_mms(den, prev_):
                # 4-way col-tiled ones-matmuls: strips 0/32 <- head A
                # (even/odd kb), strips 64/96 <- head B
                (pk0, pb0), (_pk1, pb1) = prev_
                first = pk0 == 0
                last = pk0 == TB - 2
                for hh in range(2):
                    for j, pbx in enumerate((pb0, pb1)):
                        p0 = 64 * hh + 32 * j
                        nc.tensor.matmul(
                            den[p0 : p0 + 1, :],
                            ones_sb[:, :],
                            pbx[:, hh * 512 : (hh + 1) * 512],
                            start=first,
                            stop=last,
                            tile_position=(0, p0),
                        )

            def carry_drain(carry, tail=False):
                """drain the previous pass's last attn@v/den and compute
                its 1/den rows on DVE; returns (hb, qc, (cpy, row)).

                The den strips are combined with a single partition-
                offset add: dsb[p] + dsb[32+p] puts head A's denominator
                at partition 0 and head B's at partition 64 of one tile,
                so one reciprocal + one cast serve both heads. (den's
                unwritten partitions hold 1.0 from the one-time memset,
                keeping the unused lanes finite.) The den PSUM tile is
                read by exactly one DVE op, so the next pass's den
                matmuls are released after ~0.7us."""
                acc, hbp, qcp, prev = carry
                for pkb, ppb in prev:
                    av_mms(acc, hbp, pkb, ppb)
                den_mms(den_t, prev)
                dsb = mi.tile([97, 512], F32, tag="dsb")
                if tail:
                    # scalar engine is free after the last exp: stage the
                    # den strips there so the DVE chain starts sooner
                    nc.scalar.copy(dsb[:, :], den_t[0:97, :])
                else:
                    nc.vector.tensor_copy(dsb[:, :], den_t[0:97, :])
                rows = [None]
                for hh in range(2):
                    # mixed PSUM+SBUF operands (different base partitions
                    # are only legal when not both inputs are in SBUF)
                    dsum = mi.tile([1, 512], F32, tag="dsum")
                    nc.vector.tensor_add(
                        dsum[:, :],
                        den_t[64 * hh : 64 * hh + 1, :],
                        dsb[64 * hh + 32 : 64 * hh + 33, :],
                    )
                    rec = mi.tile([1, 512], F32, tag="rec")
                    nc.vector.reciprocal_approx_fast(rec[:, :], dsum[:, :])
                    row = mi.tile([1, 512], BF16, tag="row")
                    nc.vector.tensor_copy(row[:, :], rec[:, :])
                    rows.append(row)
                # acc copy last: the 1/den rows (which gate the next
                # epi_pe's PE work) are ready as early as possible
                cpy = mi.tile([128, 512], F32, tag="cpy")
                if tail:
                    nc.scalar.copy(cpy[:, :], acc[:, :])
                else:
                    nc.vector.tensor_copy(cpy[:, :], acc[:, :])
                rows[0] = cpy
                return (hbp, qcp, rows)

            def epi_pe(hb_, qc_, rows_, psb=None):
                """broadcast 1/den to 64 partitions per head (col-tiled
                K=1 matmuls), then one DVE multiply -> attnT pair block"""
                if psb is None:
                    psb = ps2.tile(
                        [128, 512], F32, tag="psV", bufs=2,
                        name=f"psb{hb_}_{qc_}",
                    )
                for hh_ in range(2):
                    nc.tensor.matmul(
                        psb[64 * hh_ : 64 * hh_ + 64, 0:512],
                        onesK1[:, :],
                        rows_[1 + hh_][:, :],
                        start=True,
                        stop=True,
                        tile_position=(0, 64 * hh_),
                    )
                nc.vector.tensor_mul(
                    attnT_sb[:, hb_ * N + qc_ * 512 : hb_ * N + (qc_ + 1) * 512],
                    psb[0:128, 0:512],
                    rows_[0][:, :],
                )

            def emit_pass(hb, qc, carry, filler):
                """One (head pair, query quarter) pass, software-pipelined
                across passes: step-0 scores are emitted before draining
                the carried tail of the previous pass."""
                q0 = hb * N + qc * 512
                acc = ps1.tile([128, 512], F32, tag="psA", name=f"acc{hb}_{qc}")
                epi = None
                prev = []
                for step in range(8):
                    kb2 = 2 * step
                    scs = []
                    for kb in (kb2, kb2 + 1):
                        sc = ps2.tile(
                            [128, 1024], F32, tag="psS", bufs=2,
                            name=f"sc{hb}_{qc}_{kb}",
                        )
                        for hh in range(2):
                            p0 = 64 * hh
                            nc.tensor.matmul(
                                sc[:, hh * 512 : (hh + 1) * 512],
                                kT_sb[
                                    p0 : p0 + 64,
                                    hb * N + kb * 128 : hb * N + (kb + 1) * 128,
                                ],
                                qT_sb[p0 : p0 + 64, q0 : q0 + 512],
                                start=True,
                                stop=True,
                                tile_position=(p0, 0),
                            )
                        scs.append(sc)
                    # emission order matters beyond dependencies: the
                    # exps' counting-semaphore waits cover every earlier
                    # PE instruction, so anything emitted between scores
                    # and exp directly delays the ACT engine. Only the
                    # previous step's av/den (already gated by the
                    # previous exps) may precede them; the carried drain
                    # and the filler go after.
                    if prev:
                        for pkb, ppb in prev:
                            av_mms(acc, hb, pkb, ppb)
                        den_mms(den_t, prev)
                    prev = []
                    for i, kb in enumerate((kb2, kb2 + 1)):
                        pb = hp.tile([128, 1024], BF16, tag="probs")
                        nc.scalar.activation(
                            pb[:, :], scs[i][:, :], EXP, scale=SCALE
                        )
                        prev.append((kb, pb))
                    if step == 0 and carry is not None:
                        epi = carry_drain(carry)
                    if step == 3 and epi is not None:
                        epi_pe(*epi)
                        epi = None
                    filler(step)
                return (acc, hb, qc, prev)

            # ---- pre-phase ---------------------------------------------
            # persistent den accumulator; unwritten partitions hold 1.0
            # forever so the epilogue's whole-tile ops stay finite
            den_t = ps1.tile([128, 512], F32, tag="psD", name="den")
            nc.vector.memset(den_t[:, :], 1.0)
            # warm the PE clock while the first DMAs are in flight
            wtile = ps2.tile([128, 512], F32, tag="psV", bufs=2, name="warm0")
            warm_pe(wtile[0:128, :], 9)
            # first k/q blocks for head pair 0: k for tokens 0-255 only
            # (covers the first two score blocks; the rest comes in the
            # first filler slot), then the full first q quarter
            kq_tok(0, 0, False, 0, 256)
            for half in range(2):
                kq_half(0, 0, True, half)

            # ---- filler schedules -------------------------------------
            # pass 0: pair-0 v blocks JIT (2/step) + pair-0 k halves
            # early enough for scores (k tch t needed at step 2t) + q(0,1)
            def fill_pass0(step):
                if step == 0:
                    kq_tok(0, 0, False, 256, 256)
                v_unit(2 * step, 0)
                v_unit(2 * step + 1, 0)
                if step < 6:
                    kq_half(0, step // 2 + 1, False, step % 2)
                else:
                    kq_half(0, 1, True, step % 2)

            # unified filler queue for passes 1-8, in deadline order:
            # ("kq", hb, tch, is_q, half) or ("v", t128, part). Pairs
            # 1-2's v and pair 1's k/q must complete before pass 4;
            # pair 2's k/q before its first pass.
            fillq = []

            def _kq(kind, hb_, tch_):
                fillq.append(("kq", hb_, tch_, kind == "q", 0))
                fillq.append(("kq", hb_, tch_, kind == "q", 1))

            # passes 1-3 (10 items/pass): pair-1 v + pair-1 k/q. Items
            # whose DVE drain would land right at a pass boundary their
            # consumer waits on (q10 -> pass-4 scores) go early; v blocks
            # are only consumed mid-pass, so they can close a pass out.
            _kq("q", 0, 2)
            for t in range(0, 6):
                fillq.append(("v", t, 1))
            _kq("q", 0, 3)
            for t in range(6, 10):
                fillq.append(("v", t, 1))
            _kq("k", 1, 0)
            _kq("k", 1, 1)
            fillq.append(("v", 10, 1))
            fillq.append(("v", 11, 1))
            _kq("q", 1, 0)
            _kq("k", 1, 2)
            _kq("k", 1, 3)
            for t in range(12, 16):
                fillq.append(("v", t, 1))
            # passes 4-7 (8 items/pass): pair-1 q tails, pair-2 v + k/q
            _kq("q", 1, 1)
            _kq("q", 1, 2)
            _kq("q", 1, 3)
            for t in range(10):
                fillq.append(("v", t, 2))
            _kq("k", 2, 0)
            for t in range(10, 16):
                fillq.append(("v", t, 2))
            _kq("q", 2, 0)
            _kq("k", 2, 1)
            _kq("k", 2, 2)
            _kq("k", 2, 3)
            # pass 8: pair-2 q tails
            _kq("q", 2, 1)
            _kq("q", 2, 2)
            _kq("q", 2, 3)

            def fill_next():
                if fillq:
                    item = fillq.pop(0)
                    if item[0] == "v":
                        v_unit(item[1], item[2])
                    else:
                        kq_half(*item[1:])

            heavy = [1, 1, 2, 1, 1, 2, 1, 1]
            mid = [0, 1, 1, 1, 1, 2, 1, 1]
            light = [0, 1, 1, 1, 1, 1, 1, 0]

            def mk_fill_kq(counts):
                def fill_kq(step):
                    for _ in range(counts[step]):
                        fill_next()
                return fill_kq

            # passes 9-11: partial out-projection for completed quarters
            proj_queue = []

            def mk_fill_proj(counts):
                def fill_proj(step):
                    for _ in range(counts[step]):
                        if proj_queue:
                            ob_, qc_ = proj_queue.pop(0)
                            proj_unit(ob_, qc_)
                        else:
                            fill_next()
                return fill_proj

            # ---- the 12 passes ----------------------------------------
            carry = None
            for pi in range(12):
                hb, qc = divmod(pi, 4)
                if pi == 0:
                    filler = fill_pass0
                elif pi < 4:
                    filler = mk_fill_kq(heavy)
                elif pi < 8:
                    filler = mk_fill_kq(mid)
                elif pi == 8:
                    filler = mk_fill_kq(light)
                else:
                    for ob in range(CB):
                        proj_queue.append((ob, qc - 1))
                    counts = (
                        [0, 0, 0, 0, 2, 1, 1, 1]
                        if pi == 9
                        else ([0, 1, 0, 0, 2, 1, 1, 1] if pi == 10
                              else [0, 1, 0, 0, 2, 2, 1, 1])
                    )
                    filler = mk_fill_proj(counts)
                carry = emit_pass(hb, qc, carry, filler)

            # ---- tail: drain last pass + quarter-3 projection ---------
            epi = carry_drain(carry, tail=True)
            # overlap the first two proj units' pair-0/1 matmuls with the
            # DVE epilogue (keeps the PE warm); the epilogue broadcast
            # uses a scores-pool PSUM tile so both filler buffers stay
            # available for the projection chains
            psv0 = ps2.tile([128, 512], F32, tag="psV", bufs=2, name="pr0_3")
            proj_mms(0, 3, psv0, range(COB - 1))
            psv1 = ps2.tile([128, 512], F32, tag="psV", bufs=2, name="pr1_3")
            proj_mms(1, 3, psv1, range(COB - 1))
            psv2 = ps2.tile([128, 1024], F32, tag="psS", bufs=2, name="pr2_3")
            proj_mms(2, 3, psv2, range(COB - 1))
            psbt = ps2.tile([128, 1024], F32, tag="psS", bufs=2, name="psbt")
            # dummy matmuls sized to end right as the 1/den rows land,
            # then a few more to bridge the DVE multiply's latency -
            # keeps the PE clock warm without delaying the critical path
            warm_pe(psbt[0:128, 512:1024], 9)
            epi_pe(epi[0], epi[1], epi[2], psb=psbt)
            warm_pe(psbt[0:128, 512:1024], 5)
            proj_mms(0, 3, psv0, [COB - 1])
            proj_drain(0, 3, psv0, on_act=True)
            proj_mms(1, 3, psv1, [COB - 1])
            proj_drain(1, 3, psv1, on_act=True)
            proj_mms(2, 3, psv2, [COB - 1])
            proj_drain(2, 3, psv2, on_act=True)
            for ob in range(3, CB):
                proj_unit(ob, 3, on_act=True)

            wqxt[1].__exit__(None, None, None)
            wqxt[0].__exit__(None, None, None)

    nc.finalize()
    return nc


_NC_CACHE = []


def _get_nc():
    if not _NC_CACHE:
        _NC_CACHE.append(_build())
    return _NC_CACHE[0]


def kernel(x, w_qkv, w_proj, b_proj):
    x = np.asarray(x, dtype=np.float32)
    w_qkv = np.asarray(w_qkv, dtype=np.float32)
    w_proj = np.asarray(w_proj, dtype=np.float32)
    b_proj = np.asarray(b_proj, dtype=np.float32)

    nc = _get_nc()

    in_maps = []
    for core in range(N_CORES):
        b, hg = divmod(core, 2)
        # x^T packed [tch][ci][t] (identical for both cores of a batch)
        xT = x[b].T.astype(ml_dtypes.bfloat16)  # [C, N]
        xTp = np.ascontiguousarray(
            xT.reshape(CB, 128, 4, 512).transpose(1, 2, 0, 3).reshape(128, CB * N)
        )
        # own-head slice of w_qkv rows, mapped into [q | k | v] x CO space
        r0 = hg * CO
        wslice = np.concatenate(
            [
                w_qkv[r0 : r0 + CO],                  # q rows
                w_qkv[C + r0 : C + r0 + CO],          # k rows
                w_qkv[2 * C + r0 : 2 * C + r0 + CO],  # v rows
            ],
            axis=0,
        )  # [3*CO, C]
        wT = wslice.T.astype(ml_dtypes.bfloat16)  # [C, 3*CO]
        w3 = np.ascontiguousarray(wT).reshape(CB, 128, 3 * CO)
        wqp = np.concatenate(
            [
                w3[:, :, o0 : o0 + w].transpose(1, 0, 2).reshape(128, CB * w)
                for o0, w in _WQ_GROUPS
            ],
            axis=1,
        )
        wqp = np.ascontiguousarray(wqp)
        # w_proj columns for own channels, rows = c-chunks
        wp = w_proj[:, r0 : r0 + CO].T.astype(ml_dtypes.bfloat16)  # [CO, C]
        wprojp = np.ascontiguousarray(
            wp.reshape(COB, 128, C).transpose(1, 0, 2).reshape(128, COB * C)
        )
        in_maps.append({"xTp": xTp, "wqp": wqp, "wprojp": wprojp})

    res = run_bass_kernel_spmd(nc, in_maps, core_ids=list(range(N_CORES)))

    out = np.empty((B, N, C), dtype=np.float32)
    for b in range(B):
        pa = res.results[2 * b]["outT"]
        pb = res.results[2 * b + 1]["outT"]
        out[b] = (pa + pb).T + b_proj
    return out


# revision 59
# speedup vs baseline: 1.0040x; 1.0040x over previous
"""Distributed multi-head attention for TRN2 (8 NeuronCores).

Reference computation (per batch b):
    qkv = x @ w_qkv.T                         # (N, 3C)
    q, k, v = split/reshape to (H, N, D)
    attn = softmax(q @ k.T * D**-0.5)         # per head
    out = (attn @ v) reassembled to (N, C)
    out = out @ w_proj.T + b_proj

Sharding: 8 cores = 4 batches x 2 HEAD-halves (tensor parallel). Each
core computes q/k/v for its own 6 heads over all 2048 tokens (no
duplicated projection work), full attention for those heads, and a
PARTIAL output projection over its 384 channels (+ bias/2). The host
sums the two partial projections of each batch - zero cross-core comm.

Layout strategy (no on-chip transposes):
  - host passes x^T and w_qkv^T slices so projections contract over
    partitions; q,k are produced d-major; scores are computed
    transposed ([keys, queries]) which is the layout attn@v consumes
  - softmax needs no max-subtraction (scores ~ N(0,1), fp32 exp range)
  - attn@v is COLUMN-TILED: the two heads of a pair run concurrently
    in the 128x128 PE array (head A -> output partitions 0:64, head B
    -> 64:128), doubling attn@v throughput vs a 65-wide ones-column
    form
  - softmax denominators come from 4-way column-tiled ones-matmuls
    (M=1 at col strips 0/32/64/96: head x kb-parity) accumulating in
    one PSUM bank across the pass
  - normalization: DVE sums the two strip partials per head, takes the
    reciprocal, a K=1 ones-matmul broadcasts 1/den across partitions,
    and one [128,512] DVE multiply normalizes both heads at once

Schedule: ACT (softmax exp, ~1.12us per 128x1024 tile) is the
steady-state bottleneck; the whole PE schedule is built to never let
it starve. 12 passes = 3 head pairs x 4 query quarters (pair-major),
SOFTWARE-PIPELINED ACROSS PASSES: each pass emits its first scores
before draining the previous pass's last attn@v/den and epilogue, so
the ACT queue never empties at pass boundaries. Per 2-kb step: scores
alternate PE row groups; attn@v + den lag one step. PE fillers are
sized to fit the per-step exp budget (k/q units split into 3-matmul
halves): pass 0 produces v just-in-time, passes 1-8 drain k/q blocks,
passes 9-11 run the partial output projection for completed quarters.
PSUM: scores 2x2 banks + acc 1 + den 1 + filler 2 = 8.

Self-contained: hardcodes B=4, N=2048, C=768, H=12, D=64.
"""

import numpy as np
import ml_dtypes

import concourse.bass as bass
import concourse.mybir as mybir
from concourse import bacc
from concourse.tile import TileContext
from concourse.bass_utils import run_bass_kernel_spmd

F32 = mybir.dt.float32
F16 = mybir.dt.float16
BF16 = mybir.dt.bfloat16
EXP = mybir.ActivationFunctionType.Exp

B, N, C = 4, 2048, 768
H, D = 12, 64
SCALE = float(D) ** -0.5  # 0.125
HC = H // 2  # 6 heads per core
CO = HC * D  # 384 own channels
HB = HC // 2  # 3 head pairs per core
CB = C // 128  # 6 input c-chunks
COB = CO // 128  # 3 own-channel chunks
TB = N // 128  # 16 token blocks
VW = CO  # 384: v block width

N_CORES = 8

# w_qkv^T column groups (own-head slice), in consumption order:
# pair-0 k, pair-0 q, all v, then k/q for pairs 1..2. Each group holds
# its column range for all six 128-row input chunks, contiguously.
# (o0 is the offset in the 3*CO-wide own-qkv column space: q [0,CO),
# k [CO,2CO), v [2CO,3CO).)
_WQ_GROUPS = [(CO, 128), (0, 128), (2 * CO, CO)]
for _ob in range(1, COB):
    _WQ_GROUPS.append((CO + _ob * 128, 128))
    _WQ_GROUPS.append((_ob * 128, 128))
_WQ_BASE = {}
_cur = 0
for _o0, _w in _WQ_GROUPS:
    _WQ_BASE[_o0] = (_cur, _w)
    _cur += CB * _w


def _build():
    nc = bacc.Bacc(None, target_bir_lowering=False)

    xTp = nc.declare_dram_parameter("xTp", [128, CB * N], BF16, isOutput=False)
    wqp = nc.declare_dram_parameter("wqp", [128, CB * 3 * CO], BF16, isOutput=False)
    wprojp = nc.declare_dram_parameter("wprojp", [128, COB * C], BF16, isOutput=False)
    outT = nc.declare_dram_parameter("outT", [C, N], F16, isOutput=True)

    with TileContext(nc) as tc:
        with (
            tc.tile_pool(name="per", bufs=1) as per,
            tc.tile_pool(name="p23", bufs=1) as p23,
            tc.tile_pool(name="hp", bufs=8) as hp,
            tc.tile_pool(name="mi", bufs=3) as mi,
            tc.tile_pool(name="op", bufs=2) as op_pool,
            tc.tile_pool(name="ps", bufs=2, space="PSUM") as ps2,
            tc.tile_pool(name="psa", bufs=1, space="PSUM") as ps1,
        ):
            # ---- persistent tiles -------------------------------------
            qT_sb = per.tile([128, HB * N], BF16)  # q^T  [2 heads/blk, 2048]
            kT_sb = per.tile([128, HB * N], BF16)  # k^T  [2 heads/blk, 2048]
            v_sb = per.tile([128, TB * VW], BF16)  # v token-major
            ones_sb = per.tile([128, 1], BF16)  # den stationary
            onesK1 = per.tile([1, 64], BF16)  # epi broadcast stationary
            warm_sb = per.tile([128, 512], BF16)  # PE warm-up operand
            attnT_sb = p23.tile([128, HB * N], BF16)  # attn out^T
            wproj_sb = p23.tile([128, COB * C], BF16)

            nc.vector.memset(ones_sb[:, :], 1.0)
            nc.vector.memset(onesK1[:, :], 1.0)
            nc.vector.memset(warm_sb[:, :], 0.0)

            def warm_pe(psum_ap, n):
                """dummy full-array matmuls that keep the PE's activity
                monitor from re-throttling the clock across known idle
                stretches (small-K matmuls don't register as busy)"""
                for _ in range(n):
                    nc.tensor.matmul(
                        psum_ap,
                        warm_sb[:, 0:128],
                        warm_sb[:, :],
                        start=True,
                        stop=True,
                    )

            wqxt = (tc.tile_pool(name="wq", bufs=1), tc.tile_pool(name="xt", bufs=4))
            wq_pool = wqxt[0].__enter__()
            xt_pool = wqxt[1].__enter__()

            wqkv_sb = wq_pool.tile([128, CB * 3 * CO], BF16)
            xts = [
                xt_pool.tile([128, CB * 512], BF16, tag="xt", name=f"xt{t}")
                for t in range(4)
            ]

            def _dma_xt(tch):
                nc.sync.dma_start(
                    out=xts[tch][:, :],
                    in_=xTp[:, tch * CB * 512 : (tch + 1) * CB * 512],
                )

            def _dma_wq(gi):
                o0, w = _WQ_GROUPS[gi]
                base, _ = _WQ_BASE[o0]
                nc.sync.dma_start(
                    out=wqkv_sb[:, base : base + CB * w],
                    in_=wqp[:, base : base + CB * w],
                )

            def _dma_xt0_chunk(c0, c1):
                nc.sync.dma_start(
                    out=xts[0][:, c0 * 512 : c1 * 512],
                    in_=xTp[:, c0 * 512 : c1 * 512],
                )

            # consumption order: pair-0 k/q cols, then chunk 0 in pieces
            # (the pre-phase k unit consumes ci in order, so compute can
            # start while the rest of chunk 0 is still in flight), v cols
            # (JIT v units in pass 0), remaining chunks, later k/q cols
            _dma_wq(0)
            _dma_wq(1)
            _dma_xt0_chunk(0, 2)
            _dma_xt0_chunk(2, 4)
            _dma_xt0_chunk(4, 6)
            _dma_wq(2)
            for t in range(1, 4):
                _dma_xt(t)
            for gi in range(3, len(_WQ_GROUPS)):
                _dma_wq(gi)

            def wq(ci, o0, width):
                if o0 >= 2 * CO:
                    base, gw = _WQ_BASE[2 * CO]
                    off = o0 - 2 * CO
                else:
                    base, gw = _WQ_BASE[o0]
                    off = 0
                return wqkv_sb[:, base + ci * gw + off : base + ci * gw + off + width]

            nc.sync.dma_start(out=wproj_sb[:, :], in_=wprojp[:, :])

            # ---- projection work units (PE filler) --------------------
            open_kq = {}

            def kq_half(hb, tch, is_q, half):
                """half of a k^T/q^T pair-block unit (3 of 6 matmuls)"""
                key = (hb, tch, is_q)
                if half == 0:
                    open_kq[key] = ps2.tile(
                        [128, 512], F32, tag="psV", bufs=2,
                        name=f"{'q' if is_q else 'k'}{hb}_{tch}",
                    )
                psv = open_kq[key]
                for ci in range(3 * half, 3 * half + 3):
                    nc.tensor.matmul(
                        psv[:, :],
                        wq(ci, (0 if is_q else CO) + hb * 128, 128),
                        xts[tch][:, ci * 512 : (ci + 1) * 512],
                        start=(ci == 0),
                        stop=(ci == CB - 1),
                    )
                if half == 1:
                    t0 = tch * 512
                    dst = qT_sb if is_q else kT_sb
                    nc.vector.tensor_copy(
                        dst[:, hb * N + t0 : hb * N + t0 + 512], psv[:, :]
                    )
                    del open_kq[key]

            def kq_tok(hb, tch, is_q, t0, w):
                """k^T/q^T unit over a token sub-range (shorter chain so
                the first scores aren't gated on a full 512-token unit)"""
                kind = "q" if is_q else "k"
                psv = ps2.tile(
                    [128, 512], F32, tag="psV", bufs=2,
                    name=f"{kind}t{hb}_{tch}_{t0}",
                )
                for ci in range(CB):
                    nc.tensor.matmul(
                        psv[:, :w],
                        wq(ci, (0 if is_q else CO) + hb * 128, 128),
                        xts[tch][:, ci * 512 + t0 : ci * 512 + t0 + w],
                        start=(ci == 0),
                        stop=(ci == CB - 1),
                    )
                dst = qT_sb if is_q else kT_sb
                nc.vector.tensor_copy(
                    dst[:, hb * N + tch * 512 + t0 : hb * N + tch * 512 + t0 + w],
                    psv[:, :w],
                )

            def v_unit(t128, part):
                """one v block: 128 tokens x one pair's 128 v-dims.
                Pair 0 is computed just-in-time in pass 0; pair 1 in
                passes 1-3 (needed by pass 4); pair 2 in passes 4-7
                (needed by pass 8)"""
                off, w = 128 * part, 128
                tch, tb = divmod(t128, 4)
                psv = ps2.tile(
                    [128, 512], F32, tag="psV", bufs=2, name=f"v{t128}_{part}"
                )
                for ci in range(CB):
                    nc.tensor.matmul(
                        psv[:, :w],
                        xts[tch][:, ci * 512 + tb * 128 : ci * 512 + (tb + 1) * 128],
                        wq(ci, 2 * CO + off, w),
                        start=(ci == 0),
                        stop=(ci == CB - 1),
                    )
                nc.vector.tensor_copy(
                    v_sb[:, t128 * VW + off : t128 * VW + off + w], psv[:, :w]
                )

            def proj_mms(ob, qc, psv, cbs):
                for cb in cbs:
                    nc.tensor.matmul(
                        psv[0:128, 0:512],
                        wproj_sb[:, cb * C + ob * 128 : cb * C + (ob + 1) * 128],
                        attnT_sb[:, cb * N + qc * 512 : cb * N + (qc + 1) * 512],
                        start=(cb == 0),
                        stop=(cb == COB - 1),
                    )

            def proj_drain(ob, qc, psv, on_act=False):
                # fp16 partial projection out; the host adds the bias
                # while summing the two head-half partials. In the tail
                # the scalar engine is idle (no more exps), so drains go
                # there instead of the DVE, which is busy with the final
                # epilogue chain.
                ot = op_pool.tile([128, 512], F16, tag="out")
                if on_act:
                    nc.scalar.copy(ot[:, :], psv[0:128, 0:512])
                else:
                    nc.vector.tensor_copy(ot[:, :], psv[0:128, 0:512])
                nc.sync.dma_start(
                    out=outT[ob * 128 : (ob + 1) * 128, qc * 512 : (qc + 1) * 512],
                    in_=ot[:, :],
                )

            def proj_unit(ob, qc, on_act=False):
                psv = ps2.tile([128, 512], F32, tag="psV", bufs=2, name=f"pr{ob}_{qc}")
                proj_mms(ob, qc, psv, range(COB))
                proj_drain(ob, qc, psv, on_act)

            # ---- attention machinery ----------------------------------
            def av_mms(acc, hb, pkb, ppb):
                for hh in range(2):
                    vs = pkb * VW + (2 * hb + hh) * D
                    nc.tensor.matmul(
                        acc[64 * hh : 64 * hh + 64, :],
                        v_sb[:, vs : vs + D],
                        ppb[:, hh * 512 : (hh + 1) * 512],
                        start=(pkb == 0),
                        stop=(pkb == TB - 1),
                        tile_position=(0, 64 * hh),
                    )

            def den_mms(den, prev_):
                # 4-way col-tiled ones-matmuls: strips 0/32 <- head A
                # (even/odd kb), strips 64/96 <- head B
                (pk0, pb0), (_pk1, pb1) = prev_
                first = pk0 == 0
                last = pk0 == TB - 2
                for hh in range(2):
                    for j, pbx in enumerate((pb0, pb1)):
                        p0 = 64 * hh + 32 * j
                        nc.tensor.matmul(
                            den[p0 : p0 + 1, :],
                            ones_sb[:, :],
                            pbx[:, hh * 512 : (hh + 1) * 512],
                            start=first,
                            stop=last,
                            tile_position=(0, p0),
                        )

            def carry_drain(carry, tail=False):
                """drain the previous pass's last attn@v/den and compute
                its 1/den rows on DVE; returns (hb, qc, (cpy, row)).

                The den strips are combined with a single partition-
                offset add: dsb[p] + dsb[32+p] puts head A's denominator
                at partition 0 and head B's at partition 64 of one tile,
                so one reciprocal + one cast serve both heads. (den's
                unwritten partitions hold 1.0 from the one-time memset,
                keeping the unused lanes finite.) The den PSUM tile is
                read by exactly one DVE op, so the next pass's den
                matmuls are released after ~0.7us."""
                acc, hbp, qcp, prev = carry
                for pkb, ppb in prev:
                    av_mms(acc, hbp, pkb, ppb)
                den_mms(den_t, prev)
                dsb = mi.tile([97, 512], F32, tag="dsb")
                if tail:
                    # scalar engine is free after the last exp: stage the
                    # den strips there so the DVE chain starts sooner
                    nc.scalar.copy(dsb[:, :], den_t[0:97, :])
                else:
                    nc.vector.tensor_copy(dsb[:, :], den_t[0:97, :])
                rows = [None]
                for hh in range(2):
                    # mixed PSUM+SBUF operands (different base partitions
                    # are only legal when not both inputs are in SBUF)
                    dsum = mi.tile([1, 512], F32, tag="dsum")
                    nc.vector.tensor_add(
                        dsum[:, :],
                        den_t[64 * hh : 64 * hh + 1, :],
                        dsb[64 * hh + 32 : 64 * hh + 33, :],
                    )
                    rec = mi.tile([1, 512], F32, tag="rec")
                    nc.vector.reciprocal_approx_fast(rec[:, :], dsum[:, :])
                    row = mi.tile([1, 512], BF16, tag="row")
                    nc.vector.tensor_copy(row[:, :], rec[:, :])
                    rows.append(row)
                # acc copy last: the 1/den rows (which gate the next
                # epi_pe's PE work) are ready as early as possible
                cpy = mi.tile([128, 512], F32, tag="cpy")
                if tail:
                    nc.scalar.copy(cpy[:, :], acc[:, :])
                else:
                    nc.vector.tensor_copy(cpy[:, :], acc[:, :])
                rows[0] = cpy
                return (hbp, qcp, rows)

            def epi_pe(hb_, qc_, rows_, psb=None):
                """broadcast 1/den to 64 partitions per head (col-tiled
                K=1 matmuls), then one DVE multiply -> attnT pair block"""
                if psb is None:
                    psb = ps2.tile(
                        [128, 512], F32, tag="psV", bufs=2,
                        name=f"psb{hb_}_{qc_}",
                    )
                for hh_ in range(2):
                    nc.tensor.matmul(
                        psb[64 * hh_ : 64 * hh_ + 64, 0:512],
                        onesK1[:, :],
                        rows_[1 + hh_][:, :],
                        start=True,
                        stop=True,
                        tile_position=(0, 64 * hh_),
                    )
                nc.vector.tensor_mul(
                    attnT_sb[:, hb_ * N + qc_ * 512 : hb_ * N + (qc_ + 1) * 512],
                    psb[0:128, 0:512],
                    rows_[0][:, :],
                )

            def emit_pass(hb, qc, carry, filler):
                """One (head pair, query quarter) pass, software-pipelined
                across passes: step-0 scores are emitted before draining
                the carried tail of the previous pass."""
                q0 = hb * N + qc * 512
                acc = ps1.tile([128, 512], F32, tag="psA", name=f"acc{hb}_{qc}")
                epi = None
                prev = []
                for step in range(8):
                    kb2 = 2 * step
                    scs = []
                    for kb in (kb2, kb2 + 1):
                        sc = ps2.tile(
                            [128, 1024], F32, tag="psS", bufs=2,
                            name=f"sc{hb}_{qc}_{kb}",
                        )
                        for hh in range(2):
                            p0 = 64 * hh
                            nc.tensor.matmul(
                                sc[:, hh * 512 : (hh + 1) * 512],
                                kT_sb[
                                    p0 : p0 + 64,
                                    hb * N + kb * 128 : hb * N + (kb + 1) * 128,
                                ],
                                qT_sb[p0 : p0 + 64, q0 : q0 + 512],
                                start=True,
                                stop=True,
                                tile_position=(p0, 0),
                            )
                        scs.append(sc)
                    # emission order matters beyond dependencies: the
                    # exps' counting-semaphore waits cover every earlier
                    # PE instruction, so anything emitted between scores
                    # and exp directly delays the ACT engine. Only the
                    # previous step's av/den (already gated by the
                    # previous exps) may precede them; the carried drain
                    # and the filler go after.
                    if prev:
                        for pkb, ppb in prev:
                            av_mms(acc, hb, pkb, ppb)
                        den_mms(den_t, prev)
                    prev = []
                    for i, kb in enumerate((kb2, kb2 + 1)):
                        pb = hp.tile([128, 1024], BF16, tag="probs")
                        nc.scalar.activation(
                            pb[:, :], scs[i][:, :], EXP, scale=SCALE
                        )
                        prev.append((kb, pb))
                    if step == 0 and carry is not None:
                        epi = carry_drain(carry)
                    if step == 3 and epi is not None:
                        epi_pe(*epi)
                        epi = None
                    filler(step)
                return (acc, hb, qc, prev)

            # ---- pre-phase ---------------------------------------------
            # persistent den accumulator; unwritten partitions hold 1.0
            # forever so the epilogue's whole-tile ops stay finite
            den_t = ps1.tile([128, 512], F32, tag="psD", name="den")
            nc.vector.memset(den_t[:, :], 1.0)
            # warm the PE clock while the first DMAs are in flight
            wtile = ps2.tile([128, 512], F32, tag="psV", bufs=2, name="warm0")
            warm_pe(wtile[0:128, :], 9)
            # first k/q blocks for head pair 0: k for tokens 0-255 only
            # (covers the first two score blocks; the rest comes in the
            # first filler slot), then the full first q quarter
            kq_tok(0, 0, False, 0, 256)
            for half in range(2):
                kq_half(0, 0, True, half)

            # ---- filler schedules -------------------------------------
            # pass 0: pair-0 v blocks JIT (2/step) + pair-0 k halves
            # early enough for scores (k tch t needed at step 2t) + q(0,1)
            def fill_pass0(step):
                if step == 0:
                    kq_tok(0, 0, False, 256, 256)
                v_unit(2 * step, 0)
                v_unit(2 * step + 1, 0)
                if step < 6:
                    kq_half(0, step // 2 + 1, False, step % 2)
                else:
                    kq_half(0, 1, True, step % 2)

            # unified filler queue for passes 1-8, in deadline order:
            # ("kq", hb, tch, is_q, half) or ("v", t128, part). Pairs
            # 1-2's v and pair 1's k/q must complete before pass 4;
            # pair 2's k/q before its first pass.
            fillq = []

            def _kq(kind, hb_, tch_):
                fillq.append(("kq", hb_, tch_, kind == "q", 0))
                fillq.append(("kq", hb_, tch_, kind == "q", 1))

            # passes 1-3 (10 items/pass): pair-1 v + pair-1 k/q. Items
            # whose DVE drain would land right at a pass boundary their
            # consumer waits on (q10 -> pass-4 scores) go early; v blocks
            # are only consumed mid-pass, so they can close a pass out.
            _kq("q", 0, 2)
            for t in range(0, 6):
                fillq.append(("v", t, 1))
            _kq("q", 0, 3)
            for t in range(6, 10):
                fillq.append(("v", t, 1))
            _kq("k", 1, 0)
            _kq("k", 1, 1)
            fillq.append(("v", 10, 1))
            fillq.append(("v", 11, 1))
            _kq("q", 1, 0)
            _kq("k", 1, 2)
            _kq("k", 1, 3)
            for t in range(12, 16):
                fillq.append(("v", t, 1))
            # passes 4-7 (8 items/pass): pair-1 q tails, pair-2 v + k/q
            _kq("q", 1, 1)
            _kq("q", 1, 2)
            _kq("q", 1, 3)
            for t in range(10):
                fillq.append(("v", t, 2))
            _kq("k", 2, 0)
            for t in range(10, 16):
                fillq.append(("v", t, 2))
            _kq("q", 2, 0)
            _kq("k", 2, 1)
            _kq("k", 2, 2)
            _kq("k", 2, 3)
            # pass 8: pair-2 q tails
            _kq("q", 2, 1)
            _kq("q", 2, 2)
            _kq("q", 2, 3)

            def fill_next():
                if fillq:
                    item = fillq.pop(0)
                    if item[0] == "v":
                        v_unit(item[1], item[2])
                    else:
                        kq_half(*item[1:])

            heavy = [1, 1, 2, 1, 1, 2, 1, 1]
            mid = [1, 1, 1, 1, 1, 1, 1, 1]
            light = [0, 1, 1, 1, 1, 1, 1, 0]

            def mk_fill_kq(counts):
                def fill_kq(step):
                    for _ in range(counts[step]):
                        fill_next()
                return fill_kq

            # passes 9-11: partial out-projection for completed quarters
            proj_queue = []

            def mk_fill_proj(counts):
                def fill_proj(step):
                    for _ in range(counts[step]):
                        if proj_queue:
                            ob_, qc_ = proj_queue.pop(0)
                            proj_unit(ob_, qc_)
                        else:
                            fill_next()
                return fill_proj

            # ---- the 12 passes ----------------------------------------
            carry = None
            for pi in range(12):
                hb, qc = divmod(pi, 4)
                if pi == 0:
                    filler = fill_pass0
                elif pi < 4:
                    filler = mk_fill_kq(heavy)
                elif pi < 8:
                    filler = mk_fill_kq(mid)
                elif pi == 8:
                    filler = mk_fill_kq(light)
                else:
                    for ob in range(CB):
                        proj_queue.append((ob, qc - 1))
                    counts = (
                        [0, 0, 0, 1, 1, 1, 1, 1]
                        if pi == 9
                        else ([0, 1, 0, 1, 1, 1, 1, 1] if pi == 10
                              else [0, 1, 0, 1, 1, 2, 1, 1])
                    )
                    filler = mk_fill_proj(counts)
                carry = emit_pass(hb, qc, carry, filler)

            # ---- tail: drain last pass + quarter-3 projection ---------
            epi = carry_drain(carry, tail=True)
            # overlap the first two proj units' pair-0/1 matmuls with the
            # DVE epilogue (keeps the PE warm); the epilogue broadcast
            # uses a scores-pool PSUM tile so both filler buffers stay
            # available for the projection chains
            psv0 = ps2.tile([128, 512], F32, tag="psV", bufs=2, name="pr0_3")
            proj_mms(0, 3, psv0, range(COB - 1))
            psv1 = ps2.tile([128, 512], F32, tag="psV", bufs=2, name="pr1_3")
            proj_mms(1, 3, psv1, range(COB - 1))
            psv2 = ps2.tile([128, 1024], F32, tag="psS", bufs=2, name="pr2_3")
            proj_mms(2, 3, psv2, range(COB - 1))
            psbt = ps2.tile([128, 1024], F32, tag="psS", bufs=2, name="psbt")
            # dummy matmuls sized to end right as the 1/den rows land,
            # then a few more to bridge the DVE multiply's latency -
            # keeps the PE clock warm without delaying the critical path
            warm_pe(psbt[0:128, 512:1024], 9)
            epi_pe(epi[0], epi[1], epi[2], psb=psbt)
            warm_pe(psbt[0:128, 512:1024], 5)
            proj_mms(0, 3, psv0, [COB - 1])
            proj_drain(0, 3, psv0, on_act=True)
            proj_mms(1, 3, psv1, [COB - 1])
            proj_drain(1, 3, psv1, on_act=True)
            proj_mms(2, 3, psv2, [COB - 1])
            proj_drain(2, 3, psv2, on_act=True)
            for ob in range(3, CB):
                proj_unit(ob, 3, on_act=True)

            wqxt[1].__exit__(None, None, None)
            wqxt[0].__exit__(None, None, None)

    nc.finalize()
    return nc


_NC_CACHE = []


def _get_nc():
    if not _NC_CACHE:
        _NC_CACHE.append(_build())
    return _NC_CACHE[0]


def kernel(x, w_qkv, w_proj, b_proj):
    x = np.asarray(x, dtype=np.float32)
    w_qkv = np.asarray(w_qkv, dtype=np.float32)
    w_proj = np.asarray(w_proj, dtype=np.float32)
    b_proj = np.asarray(b_proj, dtype=np.float32)

    nc = _get_nc()

    in_maps = []
    for core in range(N_CORES):
        b, hg = divmod(core, 2)
        # x^T packed [tch][ci][t] (identical for both cores of a batch)
        xT = x[b].T.astype(ml_dtypes.bfloat16)  # [C, N]
        xTp = np.ascontiguousarray(
            xT.reshape(CB, 128, 4, 512).transpose(1, 2, 0, 3).reshape(128, CB * N)
        )
        # own-head slice of w_qkv rows, mapped into [q | k | v] x CO space
        r0 = hg * CO
        wslice = np.concatenate(
            [
                w_qkv[r0 : r0 + CO],                  # q rows
                w_qkv[C + r0 : C + r0 + CO],          # k rows
                w_qkv[2 * C + r0 : 2 * C + r0 + CO],  # v rows
            ],
            axis=0,
        )  # [3*CO, C]
        wT = wslice.T.astype(ml_dtypes.bfloat16)  # [C, 3*CO]
        w3 = np.ascontiguousarray(wT).reshape(CB, 128, 3 * CO)
        wqp = np.concatenate(
            [
                w3[:, :, o0 : o0 + w].transpose(1, 0, 2).reshape(128, CB * w)
                for o0, w in _WQ_GROUPS
            ],
            axis=1,
        )
        wqp = np.ascontiguousarray(wqp)
        # w_proj columns for own channels, rows = c-chunks
        wp = w_proj[:, r0 : r0 + CO].T.astype(ml_dtypes.bfloat16)  # [CO, C]
        wprojp = np.ascontiguousarray(
            wp.reshape(COB, 128, C).transpose(1, 0, 2).reshape(128, COB * C)
        )
        in_maps.append({"xTp": xTp, "wqp": wqp, "wprojp": wprojp})

    res = run_bass_kernel_spmd(nc, in_maps, core_ids=list(range(N_CORES)))

    out = np.empty((B, N, C), dtype=np.float32)
    for b in range(B):
        pa = res.results[2 * b]["outT"]
        pb = res.results[2 * b + 1]["outT"]
        out[b] = (pa + pb).T + b_proj
    return out
